# revision 58
# baseline (speedup 1.0000x reference)
"""AttentionBlock (GroupNorm32 + 8-head self-attention + proj + residual) on
8 Trainium2 NeuronCores, data-parallel over the batch (B=8 -> 1 element/core).

kernel(**inputs) takes the FULL unsharded inputs (numpy) and returns the FULL
output [8, 512, 32, 32].

Per-core device program (C=512 ch, N=1024 px, 8 heads, hd=64):
  xn  = (x - mean_g) * rsqrt(var_g + eps)        GroupNorm, gamma/beta folded
                                                 into qkv weights on the host
  q,k = Wqk xn + b   (q pre-scaled by 1/8)       [ch-on-partitions layout]
  vT  = xn^T Wv^T    (+ ones column)             [px-on-partitions layout]
  P   = exp(k_h^T q_h)  (|logits| < 7 -> no max subtraction needed)
  o,Z = vT_h^T P        (ones column of vT yields the softmax denominator Z)
  ao  = o * (1/Z)       (1/Z partition-broadcast via a DRAM bounce)
  y   = x + projW ao + proj_b'                   (v-bias folded through proj)

Key engine/schedule choices (from perfetto traces):
  - Scalar engine does exp ONLY (64 x [128,1024] tiles ~ 69us busy); its ACT
    table is preloaded by a dummy exp at t~0. GroupNorm's rstd runs on the
    DVE (rsqrt bit-trick + 2 Newton steps) so no ACT table swaps exist.
  - The PE runs ~1.2-1.4GHz (p-state); per-head PE work (P matmuls for head
    h+4 interleaved between the two AV chunks of head h) paces the kernel.
  - Weights and the qkv/vT/proj matmuls are bf16 (same PE rate as fp32r,
    half the DMA/SBUF); q/k/logits stay fp32 for softmax accuracy; P tiles
    and v are bf16.
  - 1/Z uses tensor_copy + reciprocal_approx_fast (the plain InstReciprocal
    costs 3.3us per [1,512] row!), then a DRAM-bounce partition broadcast on
    the sync queue only (scalar-queue DMAs would stall the exp stream).
  - pav PSUM tiles alternate between two pools (psC + psB) so head h+1's AV
    never waits on head h's normalize chain; PSUM = psA 4 banks (logits) +
    psB 2 + psC 2.
"""

import sys

if "/opt/trn_rl_repo" not in sys.path:
    sys.path.insert(0, "/opt/trn_rl_repo")

import numpy as np

import concourse.bass as bass
import concourse.tile as tile
from concourse import bacc, mybir
from concourse.alu_op_type import AluOpType
from concourse.bass_utils import run_bass_kernel_spmd

F32 = mybir.dt.float32
F32R = mybir.dt.float32r
BF16 = mybir.dt.bfloat16
I32 = mybir.dt.int32
AF = mybir.ActivationFunctionType

C = 512          # channels
N = 1024         # pixels (32x32)
NH = 8           # heads
HD = 64          # head dim
GS = 16          # channels per groupnorm group
EPS = 1e-5
CT = C // 128    # channel tiles
JT = N // 128    # pixel tiles
IC = N // 512    # moving chunks
NCORES = 8


def _host_prep(x, gn_gamma, gn_beta, qkv_w, qkv_b, proj_w, proj_b):
    f = np.float32
    gamma = np.asarray(gn_gamma, f)
    beta = np.asarray(gn_beta, f)
    qkv_w = np.asarray(qkv_w, f)
    qkv_b = np.asarray(qkv_b, f)
    proj_w = np.asarray(proj_w, f)
    proj_b = np.asarray(proj_b, f)
    scale = f(HD) ** f(-0.5)

    Wq, Wk, Wv = qkv_w[0:C], qkv_w[C:2 * C], qkv_w[2 * C:3 * C]
    bq = (qkv_b[0:C] + Wq @ beta) * scale
    bk = qkv_b[C:2 * C] + Wk @ beta
    bv = qkv_b[2 * C:3 * C] + Wv @ beta
    Wq = Wq * gamma[None, :] * scale
    Wk = Wk * gamma[None, :]
    Wv = Wv * gamma[None, :]

    A = np.zeros((128, 8), f)
    A[np.arange(128), np.arange(128) // GS] = f(1.0 / GS)
    E = np.zeros((8, 128), f)
    E[np.arange(128) // GS, np.arange(128)] = f(1.0)

    def to_bf16(a):
        a32 = np.ascontiguousarray(a, np.float32)
        return ((a32.view(np.uint32) + 0x8000) >> 16).astype(np.uint16)

    weights = {
        "wqkT": to_bf16(np.concatenate([Wq, Wk], 0).T),
        "wvT": to_bf16(Wv.T),
        "qkb": np.concatenate([bq, bk]).astype(f),
        "pwT": to_bf16(proj_w.T),
        "pb": (proj_b + proj_w @ bv).astype(f),
        "gA": A, "gE": E,
    }
    xs = [to_bf16(np.asarray(x[b], f).reshape(C, N))
          for b in range(x.shape[0])]
    return weights, xs


def _declare_io(nc):
    io = {}
    io["x"] = nc.dram_tensor("x", [C, N], BF16, kind="ExternalInput")
    io["wqkT"] = nc.dram_tensor("wqkT", [C, 2 * C], BF16, kind="ExternalInput")
    io["wvT"] = nc.dram_tensor("wvT", [C, C], BF16, kind="ExternalInput")
    io["qkb"] = nc.dram_tensor("qkb", [2 * C], F32, kind="ExternalInput")
    io["pwT"] = nc.dram_tensor("pwT", [C, C], BF16, kind="ExternalInput")
    io["pb"] = nc.dram_tensor("pb", [C], F32, kind="ExternalInput")
    io["gA"] = nc.dram_tensor("gA", [128, 8], F32, kind="ExternalInput")
    io["gE"] = nc.dram_tensor("gE", [8, 128], F32, kind="ExternalInput")
    io["out"] = nc.dram_tensor("out", [C, N], F32, kind="ExternalOutput")
    return io


def _build(nc, io, p_bufs=34):
    def r(ap):
        return ap.bitcast(F32R)

    with tile.TileContext(nc) as tc:
        with (
            tc.tile_pool(name="const", bufs=1) as const,
            tc.tile_pool(name="big", bufs=1) as big,
            tc.tile_pool(name="pp", bufs=p_bufs) as ppool,
            tc.tile_pool(name="sm", bufs=4) as sm,
            tc.tile_pool(name="rzp", bufs=3) as rzp,
            tc.tile_pool(name="zbp", bufs=4) as zbp,
            tc.tile_pool(name="zdp", bufs=4, space="DRAM") as zdp,
            tc.tile_pool(name="psA", bufs=2, space=bass.MemorySpace.PSUM) as psA,
            tc.tile_pool(name="psB", bufs=2, space=bass.MemorySpace.PSUM) as psB,
            tc.tile_pool(name="psC", bufs=2, space=bass.MemorySpace.PSUM) as psC,
        ):
            # ---- input loads ------------------------------------------
            # priority order x >> wqk >> wv >> pw, split over the two
            # HWDGE queues (sync + scalar); nothing early on the slow
            # SWDGE (gpsimd) queue.
            eps_sb = const.tile([128, 1], F32, tag="eps", name="eps")
            nc.vector.memset(eps_sb[:], EPS)

            x_sb = []
            for t in range(CT):
                xt = big.tile([128, N], BF16, tag=f"x{t}", name=f"x{t}")
                nc.sync.dma_start(out=xt[:, 0:512],
                                  in_=io["x"][128 * t:128 * (t + 1), 0:512])
                nc.scalar.dma_start(out=xt[:, 512:1024],
                                    in_=io["x"][128 * t:128 * (t + 1), 512:1024])
                x_sb.append(xt)

            # dummy exp: pulls the ACT table load early (after the x DMA
            # issues), off both the GN path and the first real exp
            scr = const.tile([1, 1], F32, tag="scr", name="scr")
            nc.scalar.activation(out=scr[:], in_=eps_sb[0:1, :], func=AF.Exp)

            qkb_sb = const.tile([128, 8], F32, tag="qkb", name="qkb")
            nc.scalar.dma_start(out=qkb_sb[:],
                                in_=io["qkb"][:].rearrange("(t p) -> p t", p=128))
            pb_sb = const.tile([128, 4], F32, tag="pb", name="pb")
            nc.scalar.dma_start(out=pb_sb[:],
                                in_=io["pb"][:].rearrange("(t p) -> p t", p=128))
            A_sb = const.tile([128, 8], F32, tag="gA", name="gA")
            nc.scalar.dma_start(out=A_sb[:], in_=io["gA"][:])
            E_sb = const.tile([8, 128], F32, tag="gE", name="gE")
            nc.scalar.dma_start(out=E_sb[:], in_=io["gE"][:])

            wqk_sb, wv_sb, pw_sb = [], [], []
            for t in range(CT):
                w1 = const.tile([128, 2 * C], BF16, tag=f"wqk{t}", name=f"wqk{t}")
                nc.sync.dma_start(out=w1[:],
                                  in_=io["wqkT"][128 * t:128 * (t + 1), :])
                wqk_sb.append(w1)
            for t in range(CT):
                w2 = const.tile([128, C], BF16, tag=f"wv{t}", name=f"wv{t}")
                nc.scalar.dma_start(out=w2[:],
                                    in_=io["wvT"][128 * t:128 * (t + 1), :])
                wv_sb.append(w2)
            for t in range(CT):
                w3 = const.tile([128, C], BF16, tag=f"pw{t}", name=f"pw{t}")
                nc.scalar.dma_start(out=w3[:],
                                    in_=io["pwT"][128 * t:128 * (t + 1), :])
                pw_sb.append(w3)

            ones_bf = const.tile([128, 1], BF16, tag="ones_bf", name="ones_bf")
            nc.vector.memset(ones_bf[:], 1.0)
            ones_col = const.tile([1, HD], F32, tag="ones_col", name="ones_col")
            nc.vector.memset(ones_col[:], 1.0)

            # ---- GroupNorm --------------------------------------------
            # per-channel mean / E[x^2] via bn_stats (free-dim reduction)
            stats_all = sm.tile([128, 8], F32, tag="stats_all", name="stats_all")
            sts = [sm.tile([128, 2, 6], F32, tag=f"bnst{t}", name=f"bnst{t}")
                   for t in range(CT)]
            for t in range(CT):   # left halves land first (sync queue)
                nc.vector.bn_stats(out=sts[t][:, 0, :], in_=x_sb[t][:, 0:512])
            for t in range(CT):
                nc.vector.bn_stats(out=sts[t][:, 1, :], in_=x_sb[t][:, 512:1024])
                mv = sm.tile([128, 2], F32, tag="bnmv", name="bnmv")
                nc.vector.bn_aggr(out=mv[:], in_=sts[t][:])
                nc.vector.tensor_copy(out=stats_all[:, 2 * t:2 * t + 1], in_=mv[:, 0:1])
                nc.vector.scalar_tensor_tensor(
                    out=stats_all[:, 2 * t + 1:2 * t + 2],
                    in0=mv[:, 0:1], scalar=mv[:, 0:1], in1=mv[:, 1:2],
                    op0=AluOpType.mult, op1=AluOpType.add)

            # group-aggregate across partitions with a tiny matmul
            ps_g = psB.tile([8, 8], F32, tag="ps", name="ps")
            nc.tensor.matmul(ps_g[:], lhsT=A_sb[:], rhs=stats_all[:],
                             start=True, stop=True)
            gs = sm.tile([8, 8], F32, tag="gs", name="gs")
            nc.vector.tensor_copy(out=gs[:], in_=ps_g[:])
            gsr = gs[:].rearrange("p (t s) -> p s t", s=2)
            gmean, gex2 = gsr[:, 0, :], gsr[:, 1, :]
            tmp = sm.tile([8, 2, 4], F32, tag="gtmp", name="gtmp")
            nc.vector.tensor_tensor(out=tmp[:, 0, :], in0=gmean, in1=gmean,
                                    op=AluOpType.mult)
            nc.vector.tensor_tensor(out=tmp[:, 1, :], in0=gex2, in1=tmp[:, 0, :],
                                    op=AluOpType.subtract)
            # rstd = rsqrt(var+eps) on the DVE: bit-trick seed + 2 Newton
            # steps (keeps Scalar exp-only; no ACT table swaps)
            v_t = sm.tile([8, 4], F32, tag="gv", name="gv")
            nc.vector.tensor_scalar(out=v_t[:], in0=tmp[:, 1, :],
                                    scalar1=EPS, scalar2=None,
                                    op0=AluOpType.add)
            y_t = sm.tile([8, 4], F32, tag="gy", name="gy")
            yi = y_t[:].bitcast(I32)
            nc.vector.tensor_scalar(out=yi, in0=v_t[:].bitcast(I32),
                                    scalar1=1, scalar2=None,
                                    op0=AluOpType.arith_shift_right)
            # 0x5f3759df - i  ==  (i ^ -1) + 0x5f3759e0
            nc.vector.tensor_scalar(out=yi, in0=yi,
                                    scalar1=-1, scalar2=None,
                                    op0=AluOpType.bitwise_xor)
            nc.vector.tensor_scalar(out=yi, in0=yi,
                                    scalar1=0x5F3759E0, scalar2=None,
                                    op0=AluOpType.add)
            gm = sm.tile([8, 2, 4], F32, tag="gm", name="gm")
            nw = sm.tile([8, 2, 4], F32, tag="gnw", name="gnw")
            cur = y_t[:]
            for it in range(2):
                nc.vector.tensor_tensor(out=nw[:, 0, :], in0=cur, in1=cur,
                                        op=AluOpType.mult)
                nc.vector.tensor_tensor(out=nw[:, 1, :], in0=v_t[:],
                                        in1=nw[:, 0, :], op=AluOpType.mult)
                nc.vector.tensor_scalar(out=nw[:, 1, :], in0=nw[:, 1, :],
                                        scalar1=-0.5, scalar2=1.5,
                                        op0=AluOpType.mult, op1=AluOpType.add)
                dst = gm[:, 0, :] if it == 1 else y_t[:]
                nc.vector.tensor_tensor(out=dst, in0=cur, in1=nw[:, 1, :],
                                        op=AluOpType.mult)
            nc.vector.tensor_copy(out=gm[:, 1, :], in_=gmean)

            # expand group stats back to channels (tiny matmul with E)
            ps_e = psB.tile([128, 8], F32, tag="ps", name="ps")
            nc.tensor.matmul(ps_e[:], lhsT=E_sb[:],
                             rhs=gm[:].rearrange("p s t -> p (s t)"),
                             start=True, stop=True)
            ab = sm.tile([128, 8], F32, tag="ab", name="ab")
            nc.vector.tensor_copy(out=ab[:], in_=ps_e[:])
            bvec = sm.tile([128, 4], F32, tag="bvec", name="bvec")
            nc.vector.tensor_tensor(out=bvec[:], in0=ab[:, 4:8], in1=ab[:, 0:4],
                                    op=AluOpType.mult)
            nc.vector.tensor_scalar_mul(bvec[:], bvec[:], -1.0)

            # xn = x*rstd + (-mean*rstd) in bf16, halves emitted left-first
            # so the first q/k chunk can start before all of xn is done
            xn_sb = [big.tile([128, N], BF16, tag=f"xn{t}", name=f"xn{t}")
                     for t in range(CT)]
            for half in range(2):
                for t in range(CT):
                    s = slice(512 * half, 512 * (half + 1))
                    eng = nc.vector if t < 2 else nc.gpsimd
                    eng.tensor_scalar(
                        out=xn_sb[t][:, s], in0=x_sb[t][:, s],
                        scalar1=ab[:, t:t + 1], scalar2=bvec[:, t:t + 1],
                        op0=AluOpType.mult, op1=AluOpType.add)

            # ---- q/k projections --------------------------------------
            q_sb = [big.tile([128, N], BF16, tag=f"q{t}", name=f"q{t}")
                    for t in range(CT)]
            k_sb = [big.tile([128, N], BF16, tag=f"k{t}", name=f"k{t}")
                    for t in range(CT)]

            def emit_qk(ht):
                for ot in (ht, ht + 4):      # q tile then k tile
                    dst = q_sb[ht] if ot < 4 else k_sb[ht]
                    for ic in range(IC):
                        ps = psB.tile([128, 512], F32, tag="ps", name="ps")
                        for kt in range(CT):
                            nc.tensor.matmul(
                                ps[:],
                                lhsT=wqk_sb[kt][:, 128 * ot:128 * (ot + 1)],
                                rhs=xn_sb[kt][:, 512 * ic:512 * (ic + 1)],
                                start=(kt == 0), stop=(kt == CT - 1))
                        nc.vector.tensor_scalar_add(
                            dst[:, 512 * ic:512 * (ic + 1)], ps[:],
                            qkb_sb[:, ot:ot + 1])

            # ---- P = exp(logits) in bf16 ------------------------------
            P_tiles = [[None] * JT for _ in range(NH)]

            def emit_P(h, jts):
                ht, hr = h // 2, (h % 2) * HD
                for jt in jts:
                    psp = psA.tile([128, N], F32, tag="pp", name="pp")
                    for ic in range(IC):
                        nc.tensor.matmul(
                            psp[:, 512 * ic:512 * (ic + 1)],
                            lhsT=k_sb[ht][hr:hr + HD, 128 * jt:128 * (jt + 1)],
                            rhs=q_sb[ht][hr:hr + HD, 512 * ic:512 * (ic + 1)],
                            start=True, stop=True)
                    pt = ppool.tile([128, N], BF16, tag="P", name="P")
                    nc.scalar.activation(out=pt[:], in_=psp[:], func=AF.Exp)
                    P_tiles[h][jt] = pt

            def emit_qk_chunk(ot, ic):
                ht = ot % 4
                dst = q_sb[ht] if ot < 4 else k_sb[ht]
                ps = psB.tile([128, 512], F32, tag="ps", name="ps")
                for kt in range(CT):
                    nc.tensor.matmul(
                        ps[:],
                        lhsT=wqk_sb[kt][:, 128 * ot:128 * (ot + 1)],
                        rhs=xn_sb[kt][:, 512 * ic:512 * (ic + 1)],
                        start=(kt == 0), stop=(kt == CT - 1))
                nc.vector.tensor_scalar_add(
                    dst[:, 512 * ic:512 * (ic + 1)], ps[:],
                    qkb_sb[:, ot:ot + 1])

            emit_qk_chunk(0, 0)
            emit_qk_chunk(0, 1)
            emit_qk_chunk(4, 0)
            emit_P(0, range(0, 4))
            emit_qk_chunk(4, 1)
            emit_P(0, range(4, 8))
            emit_P(1, range(JT))
            emit_qk(1)
            emit_qk(2)
            emit_qk(3)
            emit_P(2, range(JT))
            emit_P(3, range(JT))

            # ---- vT (pixel-major v + ones column for Z), bf16 ---------
            vT_sb = []
            for jt in range(JT):
                vt = big.tile([128, NH, HD + 1], BF16, tag=f"vT{jt}", name=f"vT{jt}")
                psv = psB.tile([128, 512], F32, tag="ps", name="ps")
                for kt in range(CT):
                    nc.tensor.matmul(
                        psv[:],
                        lhsT=xn_sb[kt][:, 128 * jt:128 * (jt + 1)],
                        rhs=wv_sb[kt][:],
                        start=(kt == 0), stop=(kt == CT - 1))
                nc.vector.tensor_copy(
                    out=vt[:, :, 0:HD],
                    in_=psv[:].rearrange("p (h c) -> p h c", h=NH))
                nc.vector.tensor_copy(
                    out=vt[:, :, HD:HD + 1],
                    in_=ones_bf[:].to_broadcast((128, NH, 1)))
                vT_sb.append(vt)

            # ---- attention output + softmax normalization -------------
            ao_sb = [big.tile([128, N], BF16, tag=f"ao{t}", name=f"ao{t}")
                     for t in range(CT)]

            def emit_AV_chunk(h, ic, pool):
                ht, hr = h // 2, (h % 2) * HD
                pav = pool.tile([128, 512], F32,
                                tag="pav" if pool is psC else "ps", name="pav")
                for jt in range(JT):
                    nc.tensor.matmul(
                        pav[0:HD + 1, :],
                        lhsT=vT_sb[jt][:, h, :],
                        rhs=P_tiles[h][jt][:, 512 * ic:512 * (ic + 1)],
                        start=(jt == 0), stop=(jt == JT - 1))
                # Z row -> SBUF, approx-reciprocal, partition-broadcast
                zrow = rzp.tile([1, 512], F32, tag="zrow", name="zrow")
                nc.vector.tensor_copy(out=zrow[:], in_=pav[HD:HD + 1, :])
                rz = rzp.tile([1, 512], F32, tag="rz", name="rz")
                nc.vector.reciprocal_approx_fast(out=rz[:], in_=zrow[:])
                dst = ao_sb[ht][hr:hr + HD, 512 * ic:512 * (ic + 1)]
                if h < NH - 2:
                    # DRAM bounce (latency hides under the next heads' work)
                    zd = zdp.tile([1, 512], F32, tag="zd", name="zd")
                    nc.sync.dma_start(out=zd[:], in_=rz[:])
                    zb = zbp.tile([HD, 512], F32, tag="zb", name="zb")
                    nc.sync.dma_start(out=zb[:],
                                      in_=zd[0, :].partition_broadcast(HD))
                    nc.vector.tensor_tensor(out=dst, in0=pav[0:HD, :],
                                            in1=zb[:], op=AluOpType.mult)
                else:
                    # tail heads: PE-matmul broadcast (shorter latency; the
                    # DVE mult needs one operand in SBUF, so stage pav there)
                    rz2 = rzp.tile([1, 512], F32, tag="rz2", name="rz2")
                    nc.vector.tensor_copy(out=r(rz2[:]), in_=rz[:])
                    zp = pool.tile([128, 512], F32,
                                   tag="pav" if pool is psC else "ps", name="zp")
                    nc.tensor.matmul(zp[0:HD, :], lhsT=r(ones_col[:]),
                                     rhs=r(rz2[:]), start=True, stop=True)
                    raw = zbp.tile([HD, 512], F32, tag="raw", name="raw")
                    nc.vector.tensor_copy(out=raw[:], in_=pav[0:HD, :])
                    nc.vector.tensor_tensor(out=dst, in0=raw[:],
                                            in1=zp[0:HD, :], op=AluOpType.mult)

            for h in range(NH):
                emit_AV_chunk(h, 0, psC)
                if h + 4 < NH:
                    emit_P(h + 4, range(0, 4))
                emit_AV_chunk(h, 1, psB)
                if h + 4 < NH:
                    emit_P(h + 4, range(4, 8))

            # ---- projection + bias + residual -------------------------
            # ot 0/1 prefill ct=0..2 into the psA tiles freed by the last
            # exp, so only the ct=3 matmul remains after head 7 normalizes
            pj_pre = {}
            for ot in (0, 1):
                pj = psA.tile([128, N], F32, tag="pp", name=f"pj{ot}")
                pj_pre[ot] = pj
                for ic in range(IC):
                    for ct in range(CT - 1):
                        nc.tensor.matmul(
                            pj[:, 512 * ic:512 * (ic + 1)],
                            lhsT=pw_sb[ct][:, 128 * ot:128 * (ot + 1)],
                            rhs=ao_sb[ct][:, 512 * ic:512 * (ic + 1)],
                            start=(ct == 0), stop=False)
            # ot 2/3: prefill ct=0..2 into the psB/psC rings (slots freed as
            # the last heads' normalize chains drain)
            pj_pre2 = {}
            for ot in (2, 3):
                pool2 = psB if ot == 2 else psC
                for ic in range(IC):
                    pj = pool2.tile([128, 512], F32,
                                    tag="ps" if pool2 is psB else "pav",
                                    name=f"pj{ot}_{ic}")
                    pj_pre2[(ot, ic)] = pj
                    for ct in range(CT - 1):
                        nc.tensor.matmul(
                            pj[:],
                            lhsT=pw_sb[ct][:, 128 * ot:128 * (ot + 1)],
                            rhs=ao_sb[ct][:, 512 * ic:512 * (ic + 1)],
                            start=(ct == 0), stop=False)

            out_q = [nc.scalar, nc.sync, nc.scalar, nc.sync]
            for ot in range(CT):
                y = big.tile([128, N], F32, tag=f"y{ot}", name=f"y{ot}")
                for ic in range(IC):
                    if ot in pj_pre:
                        psj = pj_pre[ot][:, 512 * ic:512 * (ic + 1)]
                        nc.tensor.matmul(
                            psj,
                            lhsT=pw_sb[CT - 1][:, 128 * ot:128 * (ot + 1)],
                            rhs=ao_sb[CT - 1][:, 512 * ic:512 * (ic + 1)],
                            start=False, stop=True)
                    else:
                        pj = pj_pre2[(ot, ic)]
                        nc.tensor.matmul(
                            pj[:],
                            lhsT=pw_sb[CT - 1][:, 128 * ot:128 * (ot + 1)],
                            rhs=ao_sb[CT - 1][:, 512 * ic:512 * (ic + 1)],
                            start=False, stop=True)
                        psj = pj[:]
                    nc.vector.scalar_tensor_tensor(
                        out=y[:, 512 * ic:512 * (ic + 1)],
                        in0=psj, scalar=pb_sb[:, ot:ot + 1],
                        in1=x_sb[ot][:, 512 * ic:512 * (ic + 1)],
                        op0=AluOpType.add, op1=AluOpType.add)
                    out_q[ot].dma_start(
                        out=io["out"][128 * ot:128 * (ot + 1),
                                      512 * ic:512 * (ic + 1)],
                        in_=y[:, 512 * ic:512 * (ic + 1)])


_NC_CACHE = {}


def _get_nc(p_bufs=34):
    key = p_bufs
    if key not in _NC_CACHE:
        nc = bacc.Bacc("TRN2", target_bir_lowering=False)
        io = _declare_io(nc)
        _build(nc, io, p_bufs=p_bufs)
        nc.compile()
        _NC_CACHE[key] = nc
    return _NC_CACHE[key]


def run(inputs, trace=False, **spmd_kwargs):
    """Build+run; returns (full_output, BassKernelResults)."""
    weights, xs = _host_prep(**inputs)
    nc = _get_nc()
    in_maps = [dict(weights, x=xs[b]) for b in range(NCORES)]
    res = run_bass_kernel_spmd(nc, in_maps, list(range(NCORES)),
                               trace=trace, **spmd_kwargs)
    out = np.stack([res.results[b]["out"].reshape(C, 32, 32)
                    for b in range(NCORES)]).astype(np.float32)
    return out, res


def kernel(**inputs):
    out, _ = run(inputs, trace=False)
    return out


if __name__ == "__main__":
    rng = np.random.default_rng(0)
    demo = {
        "x": rng.standard_normal((8, 512, 32, 32), dtype=np.float32),
        "gn_gamma": np.ones(512, np.float32),
        "gn_beta": np.zeros(512, np.float32),
        "qkv_w": rng.standard_normal((1536, 512), dtype=np.float32) / 22.6,
        "qkv_b": rng.standard_normal(1536, dtype=np.float32) * 0.02,
        "proj_w": rng.standard_normal((512, 512), dtype=np.float32) / 22.6,
        "proj_b": rng.standard_normal(512, dtype=np.float32) * 0.02,
    }
    print(kernel(**demo).shape)
